# revision 1
# baseline (speedup 1.0000x reference)
import sys
sys.path.insert(0, "/opt/trn_rl_repo")

import numpy as np
import ml_dtypes
from contextlib import ExitStack

import concourse.bass as bass
import concourse.bacc as bacc_mod
import concourse.tile as tile
import concourse.mybir as mybir
from concourse.alu_op_type import AluOpType
from concourse.bass_utils import run_bass_kernel_spmd

BF16 = mybir.dt.bfloat16
F32 = mybir.dt.float32
AF = mybir.ActivationFunctionType
AX = mybir.AxisListType

B, CIN, H, W = 4, 16, 256, 256
SHIFTS = (1, 2, 4, 8)
NS = 4          # shift heads
NH = 4          # attention heads
HID = 16
USE_COLLECTIVE = True
ROWS = 128 if USE_COLLECTIVE else 256   # rows per core
A = ROWS * W
CH = 2048       # free-dim chunk for DMA staging
NCH = A // CH
NT1 = A // 128  # pass-1 subtiles
EPS_IN = 1e-5

_OFFS = [(-1, -1), (-1, 0), (-1, 1), (0, -1), (0, 1), (1, -1), (1, 0), (1, 1)]


def _build_program():
    nc = bacc_mod.Bacc("TRN2", target_bir_lowering=False, debug=False, num_devices=8)
    sur = nc.dram_tensor("sur", [NS, 128, ROWS, W], BF16, kind="ExternalInput")
    cen = nc.dram_tensor("cen", [CIN, ROWS, W], BF16, kind="ExternalInput")
    wk = nc.dram_tensor("wk", [NS, 128, 128], BF16, kind="ExternalInput")
    wv = nc.dram_tensor("wv", [NS, 128, 128], BF16, kind="ExternalInput")
    wq = nc.dram_tensor("wq", [CIN, 64], BF16, kind="ExternalInput")
    wo = nc.dram_tensor("wo", [64, 16], BF16, kind="ExternalInput")
    bnb = nc.dram_tensor("bnb", [16, 1], F32, kind="ExternalInput")
    onesblk = nc.dram_tensor("onesblk", [64, 4], BF16, kind="ExternalInput")
    ident = nc.dram_tensor("ident", [64, 64], F32, kind="ExternalInput")
    oblkt = nc.dram_tensor("oblkt", [4, 64], F32, kind="ExternalInput")
    out = nc.dram_tensor("out", [16, ROWS, W], F32, kind="ExternalOutput")

    if USE_COLLECTIVE:
        pmask = nc.dram_tensor("pmask", [65, 8], F32, kind="ExternalInput")

    sur_f = sur.rearrange("s g r w -> s g (r w)")
    cen_f = cen.rearrange("c r w -> c (r w)")
    out_f = out.rearrange("c r w -> c (r w)")

    with tile.TileContext(nc) as tc, ExitStack() as ctx:
        singles = ctx.enter_context(tc.tile_pool(name="singles", bufs=1))
        xg_p = ctx.enter_context(tc.tile_pool(name="xg", bufs=8))
        cen_p = ctx.enter_context(tc.tile_pool(name="cen", bufs=2))
        kq_p = ctx.enter_context(tc.tile_pool(name="kq", bufs=3))
        sq_p = ctx.enter_context(tc.tile_pool(name="sq", bufs=3))
        stp = ctx.enter_context(tc.tile_pool(name="stats", bufs=1))
        vsb_p = ctx.enter_context(tc.tile_pool(name="vsb", bufs=6))
        osb_p = ctx.enter_context(tc.tile_pool(name="osb", bufs=2))
        fout_p = ctx.enter_context(tc.tile_pool(name="fout", bufs=3))
        ps1 = ctx.enter_context(ExitStack())
        ps_work = ps1.enter_context(tc.tile_pool(name="psw", bufs=2, space="PSUM"))
        ps_acc = ps1.enter_context(tc.tile_pool(name="psa", bufs=1, space="PSUM"))

        # weights to SBUF
        wk_sb = [singles.tile([128, 128], BF16, tag=f"wk{s}", name=f"wk_sb{s}") for s in range(NS)]
        wv_sb = [singles.tile([128, 128], BF16, tag=f"wv{s}", name=f"wv_sb{s}") for s in range(NS)]
        for s in range(NS):
            nc.gpsimd.dma_start(out=wk_sb[s], in_=wk[s])
            nc.gpsimd.dma_start(out=wv_sb[s], in_=wv[s])
        wq_sb = singles.tile([CIN, 64], BF16)
        nc.gpsimd.dma_start(out=wq_sb, in_=wq[:])
        wo_sb = singles.tile([64, 16], BF16)
        nc.gpsimd.dma_start(out=wo_sb, in_=wo[:])
        bnb_sb = singles.tile([16, 1], F32)
        nc.gpsimd.dma_start(out=bnb_sb, in_=bnb[:])
        oblk_sb = singles.tile([64, 4], BF16)
        nc.gpsimd.dma_start(out=oblk_sb, in_=onesblk[:])
        id_sb = singles.tile([64, 64], F32)
        nc.gpsimd.dma_start(out=id_sb, in_=ident[:])
        oblkt_sb = singles.tile([4, 64], F32)
        nc.gpsimd.dma_start(out=oblkt_sb, in_=oblkt[:])
        ones128 = singles.tile([128, 1], BF16)
        nc.vector.memset(ones128, 1.0)

        # persistent accumulators
        sc_acc = ps_acc.tile([64, 512], F32)    # scores: [64 qcols, 4s*128 kcols]
        kn_acc = ps_acc.tile([1, 512], F32)
        qn_acc = ps_acc.tile([1, 64], F32)

        # ---------------- pass 1: K,Q conv + scores + norms ----------------
        for ch in range(NCH):
            xg = []
            for s in range(NS):
                t = xg_p.tile([128, CH], BF16, tag=f"xg{s}", name=f"xgt{s}")
                nc.sync.dma_start(out=t, in_=sur_f[s, :, ch * CH:(ch + 1) * CH])
                xg.append(t)
            cen_t = cen_p.tile([CIN, CH], BF16)
            nc.sync.dma_start(out=cen_t, in_=cen_f[:, ch * CH:(ch + 1) * CH])
            for u in range(CH // 128):
                t = ch * (CH // 128) + u
                first = t == 0
                last = t == NT1 - 1
                kp = ps_work.tile([128, 512], F32, tag="kp")
                for s in range(NS):
                    nc.tensor.matmul(kp[:, s * 128:(s + 1) * 128],
                                     lhsT=xg[s][:, u * 128:(u + 1) * 128],
                                     rhs=wk_sb[s], start=True, stop=True)
                qp = ps_work.tile([128, 64], F32, tag="qp")
                nc.tensor.matmul(qp, lhsT=cen_t[:, u * 128:(u + 1) * 128],
                                 rhs=wq_sb, start=True, stop=True)
                kq = kq_p.tile([128, 576], BF16)
                nc.scalar.copy(kq[:, 0:512], kp)
                nc.scalar.copy(kq[:, 512:576], qp)
                sq = sq_p.tile([128, 576], BF16)
                nc.vector.tensor_mul(sq, kq, kq)
                for s in range(NS):
                    nc.tensor.matmul(sc_acc[:, s * 128:(s + 1) * 128],
                                     lhsT=kq[:, 512:576],
                                     rhs=kq[:, s * 128:(s + 1) * 128],
                                     start=(first and s == 0), stop=last,
                                     skip_group_check=True)
                nc.tensor.matmul(kn_acc, lhsT=ones128, rhs=sq[:, 0:512],
                                 start=first, stop=last, skip_group_check=True)
                nc.tensor.matmul(qn_acc, lhsT=ones128, rhs=sq[:, 512:576],
                                 start=first, stop=last, skip_group_check=True)

        # ---------------- stats: allreduce + attn weights ----------------
        sc_sb = stp.tile([65, 576], F32)
        nc.vector.memset(sc_sb, 0.0)
        nc.scalar.copy(sc_sb[0:64, 0:512], sc_acc)
        nc.scalar.copy(sc_sb[64:65, 0:512], kn_acc)
        nc.scalar.copy(sc_sb[64:65, 512:576], qn_acc)

        if USE_COLLECTIVE:
            pm_sb = stp.tile([65, 8], F32)
            nc.gpsimd.dma_start(out=pm_sb, in_=pmask[:])
            sti_sb = stp.tile([65, 8, 576], F32)
            for c in range(8):
                nc.vector.tensor_scalar_mul(sti_sb[:, c, :], sc_sb, pm_sb[:, c:c + 1])
            stats_full = stp.tile([65, 576], F32)
            dramp = ctx.enter_context(tc.tile_pool(name="dramp", bufs=1, space="DRAM"))
            st_in = dramp.tile([8, 65, 576], F32)
            st_out = dramp.tile([65, 576], F32)
            nc.gpsimd.dma_start(out=st_in.rearrange("s p f -> p s f"), in_=sti_sb)
            nc.gpsimd.collective_compute(
                "ReduceScatter", AluOpType.add,
                replica_groups=[[0, 1, 2, 3, 4, 5, 6, 7]],
                ins=[st_in.opt()], outs=[st_out.opt()])
            nc.gpsimd.dma_start(out=stats_full, in_=st_out[:])
        else:
            stats_full = sc_sb

        sc_raw = stats_full[0:64, 0:512]
        kn_v = stats_full[64:65, 0:512]
        qn_v = stats_full[64:65, 512:576]

        rsq = stp.tile([1, 576], F32)
        sqt = stp.tile([1, 576], F32)
        nc.scalar.activation(sqt[:, 0:512], kn_v, AF.Sqrt)
        nc.scalar.activation(sqt[:, 512:576], qn_v, AF.Sqrt, scale=float(H * W))
        nc.vector.reciprocal(rsq, sqt)
        outer_ps = ps_work.tile([64, 512], F32, tag="stx", bufs=1)
        nc.tensor.matmul(outer_ps, lhsT=rsq[:, 512:576], rhs=rsq[:, 0:512],
                         start=True, stop=True)
        outer_sb = stp.tile([64, 512], F32)
        nc.scalar.copy(outer_sb, outer_ps)
        scn = stp.tile([64, 512], F32)
        nc.vector.tensor_mul(scn, sc_raw, outer_sb)

        # gather per-head blocks: sc_g[16h2+c, s*32+j] = scn[16h2+c, s*128+32*h2+j]
        sc_g = stp.tile([64, 128], F32)
        for h2 in range(NH):
            for s in range(NS):
                nc.sync.dma_start(
                    out=sc_g[16 * h2:16 * (h2 + 1), 32 * s:32 * (s + 1)],
                    in_=scn[16 * h2:16 * (h2 + 1),
                            128 * s + 32 * h2:128 * s + 32 * h2 + 32])

        # instance-norm stats per head over [16,128] block
        sc_gb = stp.tile([64, 128], BF16)
        nc.vector.tensor_copy(sc_gb, sc_g)
        sq_gb = stp.tile([64, 128], BF16)
        nc.vector.tensor_mul(sq_gb, sc_gb, sc_gb)
        mps = ps_work.tile([4, 256], F32, tag="stx", bufs=1, name="mps")
        nc.tensor.matmul(mps[:, 0:128], lhsT=oblk_sb, rhs=sc_gb, start=True, stop=True)
        nc.tensor.matmul(mps[:, 128:256], lhsT=oblk_sb, rhs=sq_gb, start=True, stop=True)
        msums = stp.tile([4, 256], F32)
        nc.scalar.copy(msums, mps)
        sums = stp.tile([4, 2], F32)
        nc.vector.reduce_sum(sums[:, 0:1], msums[:, 0:128], axis=AX.X)
        nc.vector.reduce_sum(sums[:, 1:2], msums[:, 128:256], axis=AX.X)
        mv2 = stp.tile([4, 2], F32)
        nc.scalar.mul(mv2[:, 0:1], sums[:, 0:1], 1.0 / 2048.0)
        nc.scalar.mul(mv2[:, 1:2], sums[:, 1:2], 1.0 / 2048.0)
        m2 = stp.tile([4, 1], F32)
        nc.vector.tensor_mul(m2, mv2[:, 0:1], mv2[:, 0:1])
        var = stp.tile([4, 1], F32)
        nc.vector.tensor_sub(var, mv2[:, 1:2], m2)
        sdt = stp.tile([4, 1], F32)
        epst = stp.tile([4, 1], F32)
        nc.vector.memset(epst, EPS_IN)
        nc.scalar.activation(sdt, var, AF.Sqrt, bias=epst)
        nc.vector.reciprocal(mv2[:, 1:2], sdt)
        bc_ps = ps_work.tile([64, 2], F32, tag="stx", bufs=1, name="bc_ps")
        nc.tensor.matmul(bc_ps, lhsT=oblkt_sb, rhs=mv2, start=True, stop=True)
        bc_sb = stp.tile([64, 2], F32)
        nc.scalar.copy(bc_sb, bc_ps)
        mean_bc = bc_sb[:, 0:1]
        rstd_bc = bc_sb[:, 1:2]

        t0 = stp.tile([64, 128], F32)
        nc.vector.tensor_scalar_sub(t0, sc_g, mean_bc)
        ex = stp.tile([64, 128], F32)
        nc.scalar.activation(ex, t0, AF.Exp, scale=rstd_bc)
        rs_ = stp.tile([64, 1], F32)
        nc.vector.reduce_sum(rs_, ex, axis=AX.X)
        rr = stp.tile([64, 1], F32)
        nc.vector.reciprocal(rr, rs_)
        attn = stp.tile([64, 128], F32)
        nc.vector.tensor_scalar_mul(attn, ex, rr)

        atp = ps_work.tile([128, 64], F32, tag="stx", bufs=1, name="atp")
        nc.tensor.transpose(atp, attn, id_sb)
        attnT = stp.tile([128, 64], F32)
        nc.scalar.copy(attnT, atp)
        aw = []
        for s in range(NS):
            w = stp.tile([128, 64], BF16, tag=f"aw{s}", name=f"awt{s}")
            nc.vector.memset(w, 0.0)
            for h2 in range(NH):
                nc.vector.tensor_copy(
                    w[32 * h2:32 * h2 + 32, 16 * h2:16 * h2 + 16],
                    attnT[32 * s:32 * s + 32, 16 * h2:16 * h2 + 16])
            aw.append(w)

        # ---------------- pass 2: V conv + attn@V + outconv + BN/ReLU ----------------
        ps1.close()
        ps2 = ctx.enter_context(tc.tile_pool(name="ps2", bufs=2, space="PSUM"))
        for ch in range(NCH):
            xg = []
            for s in range(NS):
                t = xg_p.tile([128, CH], BF16, tag=f"xg{s}", name=f"xgt{s}")
                nc.sync.dma_start(out=t, in_=sur_f[s, :, ch * CH:(ch + 1) * CH])
                xg.append(t)
            for q in range(CH // 512):
                fs = 512 * q
                op = ps2.tile([64, 512], F32, tag="op")
                for s in range(NS):
                    vp = ps2.tile([128, 512], F32, tag="vp")
                    nc.tensor.matmul(vp, lhsT=wv_sb[s], rhs=xg[s][:, fs:fs + 512],
                                     start=True, stop=True)
                    vsb = vsb_p.tile([128, 512], BF16)
                    nc.vector.tensor_copy(vsb, vp)
                    nc.tensor.matmul(op, lhsT=aw[s], rhs=vsb,
                                     start=(s == 0), stop=(s == 3))
                osb = osb_p.tile([64, 512], BF16)
                nc.scalar.copy(osb, op)
                fp = ps2.tile([16, 512], F32, tag="fp")
                nc.tensor.matmul(fp, lhsT=wo_sb, rhs=osb, start=True, stop=True)
                fout = fout_p.tile([16, 512], F32)
                nc.scalar.activation(fout, fp, AF.Relu, bias=bnb_sb)
                nc.sync.dma_start(out=out_f[:, ch * CH + fs:ch * CH + fs + 512],
                                  in_=fout)
    return nc


_NC = None


def _get_nc():
    global _NC
    if _NC is None:
        _NC = _build_program()
        if not _NC.is_finalized():
            _NC.finalize()
    return _NC


def kernel(cen, q_w, k_w, v_w, out_w, bn_gamma, bn_beta, bn_mean, bn_var):
    bf = ml_dtypes.bfloat16
    pad = np.pad(cen, ((0, 0), (0, 0), (8, 8), (8, 8)), mode="reflect")  # [B,16,272,272]

    scale = bn_gamma / np.sqrt(bn_var + 1e-5)
    wo_np = (out_w * scale[:, None]).T.astype(bf)          # [64,16]
    bnb_np = (bn_beta - bn_mean * scale)[:, None].astype(np.float32)
    wq_np = np.zeros((CIN, 64), np.float32)
    for h2 in range(NH):
        for o in range(4):
            for s in range(NS):
                wq_np[:, 16 * h2 + o * 4 + s] = q_w[s, 4 * h2 + o, :]
    wq_np = wq_np.astype(bf)
    wk_np = np.ascontiguousarray(np.transpose(k_w, (0, 2, 1))).astype(bf)  # [s,128in,128out]
    wv_np = np.ascontiguousarray(np.transpose(v_w, (0, 2, 1))).astype(bf)
    oblk = np.zeros((64, 4), np.float32)
    for h2 in range(NH):
        oblk[16 * h2:16 * (h2 + 1), h2] = 1.0
    oblk = oblk.astype(bf)
    ident = np.eye(64, dtype=np.float32)

    n_cores = 8 if USE_COLLECTIVE else 8
    in_maps = []
    for core in range(n_cores):
        if USE_COLLECTIVE:
            b, half = core // 2, core % 2
            base = half * 128
        else:
            b, base = core % B, 0
        p = pad[b]  # [16, 272, 272]
        cen_loc = p[:, 8 + base:8 + base + ROWS, 8:8 + W]
        sur = np.empty((NS, 128, ROWS, W), bf)
        for s, d in enumerate(SHIFTS):
            for j, (dy, dx) in enumerate(_OFFS):
                sh = p[:, 8 + base + dy * d:8 + base + dy * d + ROWS,
                       8 + dx * d:8 + dx * d + W]
                sur[s, 16 * j:16 * (j + 1)] = (sh - cen_loc).astype(bf)
        pm = np.zeros((65, 8), np.float32)
        pm[:, 2 * (core // 2):2 * (core // 2) + 2] = 1.0
        in_maps.append(dict(
            sur=sur, cen=cen_loc.astype(bf), wk=wk_np, wv=wv_np, wq=wq_np,
            wo=wo_np, bnb=bnb_np, onesblk=oblk, ident=ident, pmask=pm,
            oblkt=np.ascontiguousarray(oblk.astype(np.float32).T)))

    res = run_bass_kernel_spmd(_get_nc(), in_maps, list(range(n_cores))).results

    out = np.empty((B, 16, H, W), np.float32)
    if USE_COLLECTIVE:
        for core in range(8):
            b, half = core // 2, core % 2
            out[b, :, half * 128:half * 128 + 128, :] = (
                res[core]["out"].reshape(16, ROWS, W))
    else:
        for b in range(B):
            out[b] = res[b]["out"].reshape(16, ROWS, W)
    return out



# revision 3
# speedup vs baseline: 21.3915x; 21.3915x over previous
import sys
sys.path.insert(0, "/opt/trn_rl_repo")

import numpy as np
import ml_dtypes
from contextlib import ExitStack

import concourse.bass as bass
import concourse.bacc as bacc_mod
import concourse.tile as tile
import concourse.mybir as mybir
from concourse.alu_op_type import AluOpType

BF16 = mybir.dt.bfloat16
F16 = mybir.dt.float16
F32 = mybir.dt.float32
AF = mybir.ActivationFunctionType
AX = mybir.AxisListType

B, CIN, H, W = 4, 16, 256, 256
SHIFTS = (1, 2, 4, 8)
NS = 4          # shift heads
NH = 4          # attention heads
HID = 16
ROWS = 128      # rows per core (B=4 images x 2 row-halves = 8 cores)
A = ROWS * W
RCH = 16        # rows per chunk
CH = RCH * W    # elements per chunk
NCH = ROWS // RCH
NT1 = A // 128  # pass-1 subtiles
EPS_IN = 1e-5
PADR, PADC = 144, 272  # per-core padded slab (128+2*8 rows, 256+2*8 cols)

_OFFS = [(-1, -1), (-1, 0), (-1, 1), (0, -1), (0, 1), (1, -1), (1, 0), (1, 1)]


def _build_program():
    nc = bacc_mod.Bacc("TRN2", target_bir_lowering=False, debug=False, num_devices=8)
    pad = nc.dram_tensor("pad", [CIN, PADR, PADC], BF16, kind="ExternalInput")
    wk = nc.dram_tensor("wk", [NS, 128, 128], BF16, kind="ExternalInput")
    wv = nc.dram_tensor("wv", [NS, 128, 128], BF16, kind="ExternalInput")
    wkc = nc.dram_tensor("wkc", [16, 512], BF16, kind="ExternalInput")
    wvc = nc.dram_tensor("wvc", [NS, 16, 128], BF16, kind="ExternalInput")
    wq = nc.dram_tensor("wq", [CIN, 64], BF16, kind="ExternalInput")
    wo = nc.dram_tensor("wo", [64, 16], BF16, kind="ExternalInput")
    bnb = nc.dram_tensor("bnb", [16, 1], F32, kind="ExternalInput")
    onesblk = nc.dram_tensor("onesblk", [64, 4], BF16, kind="ExternalInput")
    ident = nc.dram_tensor("ident", [64, 64], F32, kind="ExternalInput")
    oblkt = nc.dram_tensor("oblkt", [4, 64], F32, kind="ExternalInput")
    pmask = nc.dram_tensor("pmask", [65, 8], F32, kind="ExternalInput")
    out = nc.dram_tensor("out", [16, ROWS, W], F16, kind="ExternalOutput")

    out_f = out.rearrange("c r w -> c (r w)")

    with tile.TileContext(nc) as tc, ExitStack() as ctx:
        singles = ctx.enter_context(tc.tile_pool(name="singles", bufs=1))
        xg_p = ctx.enter_context(tc.tile_pool(name="xg", bufs=2))
        cen_p = ctx.enter_context(tc.tile_pool(name="cen", bufs=2))
        kq_p = ctx.enter_context(tc.tile_pool(name="kq", bufs=3))
        sq_p = ctx.enter_context(tc.tile_pool(name="sq", bufs=3))
        stp = ctx.enter_context(tc.tile_pool(name="stats", bufs=1))
        vsb_p = ctx.enter_context(tc.tile_pool(name="vsb", bufs=6))
        osb_p = ctx.enter_context(tc.tile_pool(name="osb", bufs=2))
        fout_p = ctx.enter_context(tc.tile_pool(name="fout", bufs=3))
        ps1 = ctx.enter_context(ExitStack())
        ps_work = ps1.enter_context(tc.tile_pool(name="psw", bufs=2, space="PSUM"))
        ps_acc = ps1.enter_context(tc.tile_pool(name="psa", bufs=1, space="PSUM"))

        # weights to SBUF
        wk_sb = [singles.tile([128, 128], BF16, tag=f"wk{s}", name=f"wk_sb{s}") for s in range(NS)]
        wv_sb = [singles.tile([128, 128], BF16, tag=f"wv{s}", name=f"wv_sb{s}") for s in range(NS)]
        wvc_sb = [singles.tile([16, 128], BF16, tag=f"wvc{s}", name=f"wvc_sb{s}") for s in range(NS)]
        for s in range(NS):
            nc.gpsimd.dma_start(out=wk_sb[s], in_=wk[s])
            nc.gpsimd.dma_start(out=wv_sb[s], in_=wv[s])
            nc.gpsimd.dma_start(out=wvc_sb[s], in_=wvc[s])
        wkc_sb = singles.tile([16, 512], BF16)
        nc.gpsimd.dma_start(out=wkc_sb, in_=wkc[:])
        wq_sb = singles.tile([CIN, 64], BF16)
        nc.gpsimd.dma_start(out=wq_sb, in_=wq[:])
        wo_sb = singles.tile([64, 16], BF16)
        nc.gpsimd.dma_start(out=wo_sb, in_=wo[:])
        bnb_sb = singles.tile([16, 1], F32)
        nc.gpsimd.dma_start(out=bnb_sb, in_=bnb[:])
        oblk_sb = singles.tile([64, 4], BF16)
        nc.gpsimd.dma_start(out=oblk_sb, in_=onesblk[:])
        id_sb = singles.tile([64, 64], F32)
        nc.gpsimd.dma_start(out=id_sb, in_=ident[:])
        oblkt_sb = singles.tile([4, 64], F32)
        nc.gpsimd.dma_start(out=oblkt_sb, in_=oblkt[:])
        ones128 = singles.tile([128, 1], BF16)
        nc.vector.memset(ones128, 1.0)

        # persistent accumulators
        sc_acc = ps_acc.tile([64, 512], F32)    # scores: [64 qcols, 4s*128 kcols]
        kn_acc = ps_acc.tile([1, 512], F32)
        qn_acc = ps_acc.tile([1, 64], F32)

        def load_chunk(ch):
            """DMA the 4x8 shifted views + the center chunk for rows
            [16*ch, 16*ch+16) straight from the padded slab."""
            r0 = RCH * ch
            xg = []
            for s in range(NS):
                d = SHIFTS[s]
                t = xg_p.tile([128, RCH, W], BF16, tag=f"xg{s}", name=f"xgt{s}")
                for j, (dy, dx) in enumerate(_OFFS):
                    eng = nc.sync if j % 2 == 0 else nc.gpsimd
                    eng.dma_start(
                        out=t[16 * j:16 * (j + 1)],
                        in_=pad[:, 8 + r0 + dy * d:8 + r0 + dy * d + RCH,
                                8 + dx * d:8 + dx * d + W])
                xg.append(t)
            cen_t = cen_p.tile([CIN, RCH, W], BF16)
            nc.sync.dma_start(out=cen_t, in_=pad[:, 8 + r0:8 + r0 + RCH, 8:8 + W])
            return xg, cen_t

        # ---------------- pass 1: K,Q conv + scores + norms ----------------
        for ch in range(NCH):
            xg, cen_t = load_chunk(ch)
            for r in range(RCH):
                for hh in range(2):
                    t = ch * (RCH * 2) + r * 2 + hh
                    first = t == 0
                    last = t == NT1 - 1
                    c0 = 128 * hh
                    # NOTE: start=True resets the whole PSUM bank, so the
                    # full-width cen matmul must come first; the per-head
                    # quarter matmuls then accumulate into their columns.
                    kp = ps_work.tile([128, 512], F32, tag="kp")
                    nc.tensor.matmul(kp, lhsT=cen_t[:, r, c0:c0 + 128],
                                     rhs=wkc_sb, start=True, stop=False,
                                     skip_group_check=True)
                    for s in range(NS):
                        nc.tensor.matmul(kp[:, s * 128:(s + 1) * 128],
                                         lhsT=xg[s][:, r, c0:c0 + 128],
                                         rhs=wk_sb[s], start=False, stop=(s == 3),
                                         skip_group_check=True)
                    qp = ps_work.tile([128, 64], F32, tag="qp")
                    nc.tensor.matmul(qp, lhsT=cen_t[:, r, c0:c0 + 128],
                                     rhs=wq_sb, start=True, stop=True)
                    kq = kq_p.tile([128, 576], BF16)
                    nc.scalar.copy(kq[:, 0:512], kp)
                    nc.scalar.copy(kq[:, 512:576], qp)
                    sq = sq_p.tile([128, 576], BF16)
                    nc.vector.tensor_mul(sq, kq, kq)
                    nc.tensor.matmul(sc_acc, lhsT=kq[:, 512:576], rhs=kq[:, 0:512],
                                     start=first, stop=last, skip_group_check=True)
                    nc.tensor.matmul(kn_acc, lhsT=ones128, rhs=sq[:, 0:512],
                                     start=first, stop=last, skip_group_check=True)
                    nc.tensor.matmul(qn_acc, lhsT=ones128, rhs=sq[:, 512:576],
                                     start=first, stop=last, skip_group_check=True)

        # ---------------- stats: allreduce + attn weights ----------------
        sc_sb = stp.tile([65, 576], F32)
        nc.vector.memset(sc_sb, 0.0)
        nc.scalar.copy(sc_sb[0:64, 0:512], sc_acc)
        nc.scalar.copy(sc_sb[64:65, 0:512], kn_acc)
        nc.scalar.copy(sc_sb[64:65, 512:576], qn_acc)

        pm_sb = stp.tile([65, 8], F32)
        nc.gpsimd.dma_start(out=pm_sb, in_=pmask[:])
        sti_sb = stp.tile([65, 8, 576], F32)
        for c in range(8):
            nc.vector.tensor_scalar_mul(sti_sb[:, c, :], sc_sb, pm_sb[:, c:c + 1])
        stats_full = stp.tile([65, 576], F32)
        dramp = ctx.enter_context(tc.tile_pool(name="dramp", bufs=1, space="DRAM"))
        st_in = dramp.tile([8, 65, 576], F32)
        st_out = dramp.tile([65, 576], F32)
        nc.gpsimd.dma_start(out=st_in.rearrange("s p f -> p s f"), in_=sti_sb)
        nc.gpsimd.collective_compute(
            "ReduceScatter", AluOpType.add,
            replica_groups=[[0, 1, 2, 3, 4, 5, 6, 7]],
            ins=[st_in.opt()], outs=[st_out.opt()])
        nc.gpsimd.dma_start(out=stats_full, in_=st_out[:])

        sc_raw = stats_full[0:64, 0:512]
        kn_v = stats_full[64:65, 0:512]
        qn_v = stats_full[64:65, 512:576]

        rsq = stp.tile([1, 576], F32)
        sqt = stp.tile([1, 576], F32)
        nc.scalar.activation(sqt[:, 0:512], kn_v, AF.Sqrt)
        nc.scalar.activation(sqt[:, 512:576], qn_v, AF.Sqrt, scale=float(H * W))
        nc.vector.reciprocal(rsq, sqt)
        outer_ps = ps_work.tile([64, 512], F32, tag="stx", bufs=1)
        nc.tensor.matmul(outer_ps, lhsT=rsq[:, 512:576], rhs=rsq[:, 0:512],
                         start=True, stop=True)
        outer_sb = stp.tile([64, 512], F32)
        nc.scalar.copy(outer_sb, outer_ps)
        scn = stp.tile([64, 512], F32)
        nc.vector.tensor_mul(scn, sc_raw, outer_sb)

        # gather per-head blocks: sc_g[16h2+c, s*32+j] = scn[16h2+c, s*128+32*h2+j]
        sc_g = stp.tile([64, 128], F32)
        for h2 in range(NH):
            for s in range(NS):
                nc.sync.dma_start(
                    out=sc_g[16 * h2:16 * (h2 + 1), 32 * s:32 * (s + 1)],
                    in_=scn[16 * h2:16 * (h2 + 1),
                            128 * s + 32 * h2:128 * s + 32 * h2 + 32])

        # instance-norm stats per head over [16,128] block
        sc_gb = stp.tile([64, 128], BF16)
        nc.vector.tensor_copy(sc_gb, sc_g)
        sq_gb = stp.tile([64, 128], BF16)
        nc.vector.tensor_mul(sq_gb, sc_gb, sc_gb)
        mps = ps_work.tile([4, 256], F32, tag="stx", bufs=1, name="mps")
        nc.tensor.matmul(mps[:, 0:128], lhsT=oblk_sb, rhs=sc_gb, start=True, stop=True)
        nc.tensor.matmul(mps[:, 128:256], lhsT=oblk_sb, rhs=sq_gb, start=True, stop=True)
        msums = stp.tile([4, 256], F32)
        nc.scalar.copy(msums, mps)
        sums = stp.tile([4, 2], F32)
        nc.vector.reduce_sum(sums[:, 0:1], msums[:, 0:128], axis=AX.X)
        nc.vector.reduce_sum(sums[:, 1:2], msums[:, 128:256], axis=AX.X)
        mv2 = stp.tile([4, 2], F32)
        nc.scalar.mul(mv2[:, 0:1], sums[:, 0:1], 1.0 / 2048.0)
        nc.scalar.mul(mv2[:, 1:2], sums[:, 1:2], 1.0 / 2048.0)
        m2 = stp.tile([4, 1], F32)
        nc.vector.tensor_mul(m2, mv2[:, 0:1], mv2[:, 0:1])
        var = stp.tile([4, 1], F32)
        nc.vector.tensor_sub(var, mv2[:, 1:2], m2)
        sdt = stp.tile([4, 1], F32)
        epst = stp.tile([4, 1], F32)
        nc.vector.memset(epst, EPS_IN)
        nc.scalar.activation(sdt, var, AF.Sqrt, bias=epst)
        nc.vector.reciprocal(mv2[:, 1:2], sdt)
        bc_ps = ps_work.tile([64, 2], F32, tag="stx", bufs=1, name="bc_ps")
        nc.tensor.matmul(bc_ps, lhsT=oblkt_sb, rhs=mv2, start=True, stop=True)
        bc_sb = stp.tile([64, 2], F32)
        nc.scalar.copy(bc_sb, bc_ps)
        mean_bc = bc_sb[:, 0:1]
        rstd_bc = bc_sb[:, 1:2]

        t0 = stp.tile([64, 128], F32)
        nc.vector.tensor_scalar_sub(t0, sc_g, mean_bc)
        ex = stp.tile([64, 128], F32)
        nc.scalar.activation(ex, t0, AF.Exp, scale=rstd_bc)
        rs_ = stp.tile([64, 1], F32)
        nc.vector.reduce_sum(rs_, ex, axis=AX.X)
        rr = stp.tile([64, 1], F32)
        nc.vector.reciprocal(rr, rs_)
        attn = stp.tile([64, 128], F32)
        nc.vector.tensor_scalar_mul(attn, ex, rr)

        atp = ps_work.tile([128, 64], F32, tag="stx", bufs=1, name="atp")
        nc.tensor.transpose(atp, attn, id_sb)
        attnT = stp.tile([128, 64], F32)
        nc.scalar.copy(attnT, atp)
        aw = []
        for s in range(NS):
            w = stp.tile([128, 64], BF16, tag=f"aw{s}", name=f"awt{s}")
            nc.vector.memset(w, 0.0)
            for h2 in range(NH):
                nc.vector.tensor_copy(
                    w[32 * h2:32 * h2 + 32, 16 * h2:16 * h2 + 16],
                    attnT[32 * s:32 * s + 32, 16 * h2:16 * h2 + 16])
            aw.append(w)

        # ---------------- pass 2: V conv + attn@V + outconv + BN/ReLU ----------------
        ps1.close()
        ps2 = ctx.enter_context(tc.tile_pool(name="ps2", bufs=2, space="PSUM"))
        for ch in range(NCH):
            xg, cen_t = load_chunk(ch)
            for q in range(CH // 512):
                fs = 512 * q
                op = ps2.tile([64, 512], F32, tag="op")
                for s in range(NS):
                    vp = ps2.tile([128, 512], F32, tag="vp")
                    nc.tensor.matmul(vp, lhsT=wv_sb[s], rhs=xg[s][:, 2 * q:2 * q + 2, :],
                                     start=True, stop=False, skip_group_check=True)
                    nc.tensor.matmul(vp, lhsT=wvc_sb[s], rhs=cen_t[:, 2 * q:2 * q + 2, :],
                                     start=False, stop=True, skip_group_check=True)
                    vsb = vsb_p.tile([128, 512], BF16)
                    nc.vector.tensor_copy(vsb, vp)
                    nc.tensor.matmul(op, lhsT=aw[s], rhs=vsb,
                                     start=(s == 0), stop=(s == 3))
                osb = osb_p.tile([64, 512], BF16)
                nc.scalar.copy(osb, op)
                fp = ps2.tile([16, 512], F32, tag="fp")
                nc.tensor.matmul(fp, lhsT=wo_sb, rhs=osb, start=True, stop=True)
                fout = fout_p.tile([16, 512], F16)
                nc.scalar.activation(fout, fp, AF.Relu, bias=bnb_sb)
                nc.sync.dma_start(out=out_f[:, ch * CH + fs:ch * CH + fs + 512],
                                  in_=fout)
    return nc


_RT = None


def _get_rt():
    """Build the Bass program once and wrap it in a cached jitted SPMD
    executable (same machinery as bass_utils.run_bass_kernel_spmd's axon
    path, but the jit wrapper is reused across calls so recompilation
    happens only once per process)."""
    global _RT
    if _RT is not None:
        return _RT
    import jax
    from jax.sharding import Mesh, PartitionSpec
    from jax.experimental.shard_map import shard_map
    from concourse import bass2jax as b2j

    nc = _build_program()
    if not nc.is_finalized():
        nc.finalize()
    b2j.install_neuronx_cc_hook()
    partition_name = nc.partition_id_tensor.name if nc.partition_id_tensor else None
    in_names, out_names, out_avals, zero_shapes = [], [], [], []
    for alloc in nc.m.functions[0].allocations:
        if not isinstance(alloc, mybir.MemoryLocationSet):
            continue
        name = alloc.memorylocations[0].name
        if alloc.kind == "ExternalInput":
            if name != partition_name:
                in_names.append(name)
        elif alloc.kind == "ExternalOutput":
            shape = tuple(alloc.tensor_shape)
            dtype = mybir.dt.np(alloc.dtype)
            out_avals.append(jax.core.ShapedArray(shape, dtype))
            out_names.append(name)
            zero_shapes.append((shape, dtype))
    n_params = len(in_names)
    n_outs = len(out_avals)
    all_in = list(in_names) + list(out_names)
    if partition_name is not None:
        all_in.append(partition_name)
    donate = tuple(range(n_params, n_params + n_outs))

    def _body(*args):
        operands = list(args)
        if partition_name is not None:
            operands.append(b2j.partition_id_tensor())
        outs = b2j._bass_exec_p.bind(
            *operands,
            out_avals=tuple(out_avals),
            in_names=tuple(all_in),
            out_names=tuple(out_names),
            lowering_input_output_aliases=(),
            sim_require_finite=True,
            sim_require_nnan=True,
            nc=nc,
        )
        return tuple(outs)

    devices = jax.devices()[:8]
    mesh = Mesh(np.asarray(devices), ("core",))
    in_specs = (PartitionSpec("core"),) * (n_params + n_outs)
    out_specs = (PartitionSpec("core"),) * n_outs
    sharded = jax.jit(
        shard_map(_body, mesh=mesh, in_specs=in_specs,
                  out_specs=out_specs, check_rep=False),
        donate_argnums=donate, keep_unused=True)
    _RT = (sharded, in_names, out_names, out_avals, zero_shapes)
    return _RT


def kernel(cen, q_w, k_w, v_w, out_w, bn_gamma, bn_beta, bn_mean, bn_var):
    bf = ml_dtypes.bfloat16
    sharded, in_names, out_names, out_avals, zero_shapes = _get_rt()

    # reflect-pad once, cast to bf16 once: [B, 16, 272, 272]
    padded = np.pad(cen, ((0, 0), (0, 0), (8, 8), (8, 8)), mode="reflect").astype(bf)

    scale = bn_gamma / np.sqrt(bn_var + 1e-5)
    wo_np = (out_w * scale[:, None]).T.astype(bf)          # [64,16]
    bnb_np = (bn_beta - bn_mean * scale)[:, None].astype(np.float32)
    wq_np = np.zeros((CIN, 64), np.float32)
    for h2 in range(NH):
        for o in range(4):
            for s in range(NS):
                wq_np[:, 16 * h2 + o * 4 + s] = q_w[s, 4 * h2 + o, :]
    wq_np = wq_np.astype(bf)
    wk_np = np.ascontiguousarray(np.transpose(k_w, (0, 2, 1))).astype(bf)  # [s,128in,128out]
    wv_np = np.ascontiguousarray(np.transpose(v_w, (0, 2, 1))).astype(bf)
    # folded center-subtraction weights: sur = shift - cen, so
    # K = Wk@shift + (-sum_j Wk_j)@cen; same for V.
    wkc_np = np.concatenate(
        [-k_w[s].reshape(128, 8, 16).sum(axis=1).T for s in range(NS)],
        axis=1).astype(bf)                                  # [16, 512]
    wvc_np = np.stack(
        [-v_w[s].reshape(128, 8, 16).sum(axis=1).T for s in range(NS)]).astype(bf)
    oblk = np.zeros((64, 4), np.float32)
    for h2 in range(NH):
        oblk[16 * h2:16 * (h2 + 1), h2] = 1.0
    oblk = oblk.astype(bf)
    ident = np.eye(64, dtype=np.float32)
    oblkt = np.ascontiguousarray(oblk.astype(np.float32).T)

    per_core = {name: [] for name in in_names}
    for core in range(8):
        b, half = core // 2, core % 2
        base = half * 128
        pm = np.zeros((65, 8), np.float32)
        pm[:, 2 * (core // 2):2 * (core // 2) + 2] = 1.0
        vals = dict(
            pad=padded[b, :, base:base + PADR, :], wk=wk_np, wv=wv_np,
            wkc=wkc_np, wvc=wvc_np, wq=wq_np, wo=wo_np, bnb=bnb_np,
            onesblk=oblk, ident=ident, oblkt=oblkt, pmask=pm)
        for name in in_names:
            per_core[name].append(vals[name])
    concat_in = [np.concatenate(per_core[name], axis=0) for name in in_names]
    concat_zeros = [np.zeros((8 * shape[0], *shape[1:]), dtype)
                    for shape, dtype in zero_shapes]

    out_arrs = sharded(*concat_in, *concat_zeros)
    res = np.asarray(out_arrs[out_names.index("out")]).reshape(8, 16, ROWS, W)

    out = np.empty((B, 16, H, W), np.float32)
    for core in range(8):
        b, half = core // 2, core % 2
        out[b, :, half * 128:half * 128 + 128, :] = res[core].astype(np.float32)
    return out


# revision 4
# speedup vs baseline: 23.7610x; 1.1108x over previous
import sys
sys.path.insert(0, "/opt/trn_rl_repo")

import numpy as np
import ml_dtypes
from contextlib import ExitStack

import concourse.bass as bass
import concourse.bacc as bacc_mod
import concourse.tile as tile
import concourse.mybir as mybir
from concourse.alu_op_type import AluOpType

BF16 = mybir.dt.bfloat16
F16 = mybir.dt.float16
F32 = mybir.dt.float32
AF = mybir.ActivationFunctionType
AX = mybir.AxisListType

B, CIN, H, W = 4, 16, 256, 256
SHIFTS = (1, 2, 4, 8)
NS = 4          # shift heads
NH = 4          # attention heads
HID = 16
ROWS = 128      # rows per core (B=4 images x 2 row-halves = 8 cores)
A = ROWS * W
RCH = 16        # rows per chunk
CH = RCH * W    # elements per chunk
NCH = ROWS // RCH
NT1 = A // 128  # pass-1 subtiles
EPS_IN = 1e-5
PADR, PADC = 144, 272  # per-core padded slab (128+2*8 rows, 256+2*8 cols)

_OFFS = [(-1, -1), (-1, 0), (-1, 1), (0, -1), (0, 1), (1, -1), (1, 0), (1, 1)]


def _build_program():
    nc = bacc_mod.Bacc("TRN2", target_bir_lowering=False, debug=False, num_devices=8)
    pad = nc.dram_tensor("pad", [CIN, PADR, PADC], BF16, kind="ExternalInput")
    wk = nc.dram_tensor("wk", [NS, 128, 128], BF16, kind="ExternalInput")
    wv = nc.dram_tensor("wv", [NS, 128, 128], BF16, kind="ExternalInput")
    wkc = nc.dram_tensor("wkc", [16, 512], BF16, kind="ExternalInput")
    wvc = nc.dram_tensor("wvc", [NS, 16, 128], BF16, kind="ExternalInput")
    wq = nc.dram_tensor("wq", [CIN, 64], BF16, kind="ExternalInput")
    wo = nc.dram_tensor("wo", [64, 16], BF16, kind="ExternalInput")
    bnb = nc.dram_tensor("bnb", [16, 1], F32, kind="ExternalInput")
    onesblk = nc.dram_tensor("onesblk", [64, 4], BF16, kind="ExternalInput")
    ident = nc.dram_tensor("ident", [64, 64], F32, kind="ExternalInput")
    oblkt = nc.dram_tensor("oblkt", [4, 64], F32, kind="ExternalInput")
    pmask = nc.dram_tensor("pmask", [65, 8], F32, kind="ExternalInput")
    out = nc.dram_tensor("out", [16, ROWS, W], F16, kind="ExternalOutput")

    out_f = out.rearrange("c r w -> c (r w)")

    with tile.TileContext(nc) as tc, ExitStack() as ctx:
        singles = ctx.enter_context(tc.tile_pool(name="singles", bufs=1))
        xg_p = ctx.enter_context(tc.tile_pool(name="xg", bufs=2))
        cen_p = ctx.enter_context(tc.tile_pool(name="cen", bufs=2))
        kq_p = ctx.enter_context(tc.tile_pool(name="kq", bufs=3))
        sq_p = ctx.enter_context(tc.tile_pool(name="sq", bufs=3))
        stp = ctx.enter_context(tc.tile_pool(name="stats", bufs=1))
        vsb_p = ctx.enter_context(tc.tile_pool(name="vsb", bufs=6))
        osb_p = ctx.enter_context(tc.tile_pool(name="osb", bufs=2))
        fout_p = ctx.enter_context(tc.tile_pool(name="fout", bufs=3))
        ps1 = ctx.enter_context(ExitStack())
        ps_work = ps1.enter_context(tc.tile_pool(name="psw", bufs=2, space="PSUM"))
        ps_acc = ps1.enter_context(tc.tile_pool(name="psa", bufs=1, space="PSUM"))

        # weights to SBUF
        wk_sb = [singles.tile([128, 128], BF16, tag=f"wk{s}", name=f"wk_sb{s}") for s in range(NS)]
        wv_sb = [singles.tile([128, 128], BF16, tag=f"wv{s}", name=f"wv_sb{s}") for s in range(NS)]
        wvc_sb = [singles.tile([16, 128], BF16, tag=f"wvc{s}", name=f"wvc_sb{s}") for s in range(NS)]
        for s in range(NS):
            nc.gpsimd.dma_start(out=wk_sb[s], in_=wk[s])
            nc.gpsimd.dma_start(out=wv_sb[s], in_=wv[s])
            nc.gpsimd.dma_start(out=wvc_sb[s], in_=wvc[s])
        wkc_sb = singles.tile([16, 512], BF16)
        nc.gpsimd.dma_start(out=wkc_sb, in_=wkc[:])
        wq_sb = singles.tile([CIN, 64], BF16)
        nc.gpsimd.dma_start(out=wq_sb, in_=wq[:])
        wo_sb = singles.tile([64, 16], BF16)
        nc.gpsimd.dma_start(out=wo_sb, in_=wo[:])
        bnb_sb = singles.tile([16, 1], F32)
        nc.gpsimd.dma_start(out=bnb_sb, in_=bnb[:])
        oblk_sb = singles.tile([64, 4], BF16)
        nc.gpsimd.dma_start(out=oblk_sb, in_=onesblk[:])
        id_sb = singles.tile([64, 64], F32)
        nc.gpsimd.dma_start(out=id_sb, in_=ident[:])
        oblkt_sb = singles.tile([4, 64], F32)
        nc.gpsimd.dma_start(out=oblkt_sb, in_=oblkt[:])
        ones128 = singles.tile([128, 1], BF16)
        nc.vector.memset(ones128, 1.0)

        # persistent accumulators
        sc_acc = ps_acc.tile([64, 512], F32)    # scores: [64 qcols, 4s*128 kcols]
        kn_acc = ps_acc.tile([1, 512], F32)
        qn_acc = ps_acc.tile([1, 64], F32)

        def load_chunk(ch):
            """DMA the 4x8 shifted views + the center chunk for rows
            [16*ch, 16*ch+16) straight from the padded slab."""
            r0 = RCH * ch
            xg = []
            for s in range(NS):
                d = SHIFTS[s]
                t = xg_p.tile([128, RCH, W], BF16, tag=f"xg{s}", name=f"xgt{s}")
                for j, (dy, dx) in enumerate(_OFFS):
                    eng = nc.sync if j % 2 == 0 else nc.gpsimd
                    eng.dma_start(
                        out=t[16 * j:16 * (j + 1)],
                        in_=pad[:, 8 + r0 + dy * d:8 + r0 + dy * d + RCH,
                                8 + dx * d:8 + dx * d + W])
                xg.append(t)
            cen_t = cen_p.tile([CIN, RCH, W], BF16)
            nc.sync.dma_start(out=cen_t, in_=pad[:, 8 + r0:8 + r0 + RCH, 8:8 + W])
            return xg, cen_t

        # ---------------- pass 1: K,Q conv + scores + norms ----------------
        for ch in range(NCH):
            xg, cen_t = load_chunk(ch)
            for r in range(RCH):
                for hh in range(2):
                    t = ch * (RCH * 2) + r * 2 + hh
                    first = t == 0
                    last = t == NT1 - 1
                    c0 = 128 * hh
                    # NOTE: start=True resets the whole PSUM bank, so the
                    # full-width cen matmul must come first; the per-head
                    # quarter matmuls then accumulate into their columns.
                    kp = ps_work.tile([128, 512], F32, tag="kp")
                    nc.tensor.matmul(kp, lhsT=cen_t[:, r, c0:c0 + 128],
                                     rhs=wkc_sb, start=True, stop=False,
                                     skip_group_check=True)
                    for s in range(NS):
                        nc.tensor.matmul(kp[:, s * 128:(s + 1) * 128],
                                         lhsT=xg[s][:, r, c0:c0 + 128],
                                         rhs=wk_sb[s], start=False, stop=(s == 3),
                                         skip_group_check=True)
                    qp = ps_work.tile([128, 64], F32, tag="qp")
                    nc.tensor.matmul(qp, lhsT=cen_t[:, r, c0:c0 + 128],
                                     rhs=wq_sb, start=True, stop=True)
                    kq = kq_p.tile([128, 576], BF16)
                    nc.scalar.copy(kq[:, 0:512], kp)
                    nc.scalar.copy(kq[:, 512:576], qp)
                    sq = sq_p.tile([128, 576], BF16)
                    nc.vector.tensor_mul(sq, kq, kq)
                    nc.tensor.matmul(sc_acc, lhsT=kq[:, 512:576], rhs=kq[:, 0:512],
                                     start=first, stop=last, skip_group_check=True)
                    nc.tensor.matmul(kn_acc, lhsT=ones128, rhs=sq[:, 0:512],
                                     start=first, stop=last, skip_group_check=True)
                    nc.tensor.matmul(qn_acc, lhsT=ones128, rhs=sq[:, 512:576],
                                     start=first, stop=last, skip_group_check=True)

        # ---------------- stats: allreduce + attn weights ----------------
        sc_sb = stp.tile([65, 576], F32)
        nc.vector.memset(sc_sb, 0.0)
        nc.scalar.copy(sc_sb[0:64, 0:512], sc_acc)
        nc.scalar.copy(sc_sb[64:65, 0:512], kn_acc)
        nc.scalar.copy(sc_sb[64:65, 512:576], qn_acc)

        pm_sb = stp.tile([65, 8], F32)
        nc.gpsimd.dma_start(out=pm_sb, in_=pmask[:])
        sti_sb = stp.tile([65, 8, 576], F32)
        for c in range(8):
            nc.vector.tensor_scalar_mul(sti_sb[:, c, :], sc_sb, pm_sb[:, c:c + 1])
        stats_full = stp.tile([65, 576], F32)
        dramp = ctx.enter_context(tc.tile_pool(name="dramp", bufs=1, space="DRAM"))
        st_in = dramp.tile([8, 65, 576], F32)
        st_out = dramp.tile([65, 576], F32)
        nc.gpsimd.dma_start(out=st_in.rearrange("s p f -> p s f"), in_=sti_sb)
        nc.gpsimd.collective_compute(
            "ReduceScatter", AluOpType.add,
            replica_groups=[[0, 1, 2, 3, 4, 5, 6, 7]],
            ins=[st_in.opt()], outs=[st_out.opt()])
        nc.gpsimd.dma_start(out=stats_full, in_=st_out[:])

        sc_raw = stats_full[0:64, 0:512]
        kn_v = stats_full[64:65, 0:512]
        qn_v = stats_full[64:65, 512:576]

        rsq = stp.tile([1, 576], F32)
        sqt = stp.tile([1, 576], F32)
        nc.scalar.activation(sqt[:, 0:512], kn_v, AF.Sqrt)
        nc.scalar.activation(sqt[:, 512:576], qn_v, AF.Sqrt, scale=float(H * W))
        nc.vector.reciprocal(rsq, sqt)
        outer_ps = ps_work.tile([64, 512], F32, tag="stx", bufs=1)
        nc.tensor.matmul(outer_ps, lhsT=rsq[:, 512:576], rhs=rsq[:, 0:512],
                         start=True, stop=True)
        outer_sb = stp.tile([64, 512], F32)
        nc.scalar.copy(outer_sb, outer_ps)
        scn = stp.tile([64, 512], F32)
        nc.vector.tensor_mul(scn, sc_raw, outer_sb)

        # gather per-head blocks: sc_g[16h2+c, s*32+j] = scn[16h2+c, s*128+32*h2+j]
        sc_g = stp.tile([64, 128], F32)
        for h2 in range(NH):
            for s in range(NS):
                nc.sync.dma_start(
                    out=sc_g[16 * h2:16 * (h2 + 1), 32 * s:32 * (s + 1)],
                    in_=scn[16 * h2:16 * (h2 + 1),
                            128 * s + 32 * h2:128 * s + 32 * h2 + 32])

        # instance-norm stats per head over [16,128] block
        sc_gb = stp.tile([64, 128], BF16)
        nc.vector.tensor_copy(sc_gb, sc_g)
        sq_gb = stp.tile([64, 128], BF16)
        nc.vector.tensor_mul(sq_gb, sc_gb, sc_gb)
        mps = ps_work.tile([4, 256], F32, tag="stx", bufs=1, name="mps")
        nc.tensor.matmul(mps[:, 0:128], lhsT=oblk_sb, rhs=sc_gb, start=True, stop=True)
        nc.tensor.matmul(mps[:, 128:256], lhsT=oblk_sb, rhs=sq_gb, start=True, stop=True)
        msums = stp.tile([4, 256], F32)
        nc.scalar.copy(msums, mps)
        sums = stp.tile([4, 2], F32)
        nc.vector.reduce_sum(sums[:, 0:1], msums[:, 0:128], axis=AX.X)
        nc.vector.reduce_sum(sums[:, 1:2], msums[:, 128:256], axis=AX.X)
        mv2 = stp.tile([4, 2], F32)
        nc.scalar.mul(mv2[:, 0:1], sums[:, 0:1], 1.0 / 2048.0)
        nc.scalar.mul(mv2[:, 1:2], sums[:, 1:2], 1.0 / 2048.0)
        m2 = stp.tile([4, 1], F32)
        nc.vector.tensor_mul(m2, mv2[:, 0:1], mv2[:, 0:1])
        var = stp.tile([4, 1], F32)
        nc.vector.tensor_sub(var, mv2[:, 1:2], m2)
        sdt = stp.tile([4, 1], F32)
        epst = stp.tile([4, 1], F32)
        nc.vector.memset(epst, EPS_IN)
        nc.scalar.activation(sdt, var, AF.Sqrt, bias=epst)
        nc.vector.reciprocal(mv2[:, 1:2], sdt)
        bc_ps = ps_work.tile([64, 2], F32, tag="stx", bufs=1, name="bc_ps")
        nc.tensor.matmul(bc_ps, lhsT=oblkt_sb, rhs=mv2, start=True, stop=True)
        bc_sb = stp.tile([64, 2], F32)
        nc.scalar.copy(bc_sb, bc_ps)
        mean_bc = bc_sb[:, 0:1]
        rstd_bc = bc_sb[:, 1:2]

        t0 = stp.tile([64, 128], F32)
        nc.vector.tensor_scalar_sub(t0, sc_g, mean_bc)
        ex = stp.tile([64, 128], F32)
        nc.scalar.activation(ex, t0, AF.Exp, scale=rstd_bc)
        rs_ = stp.tile([64, 1], F32)
        nc.vector.reduce_sum(rs_, ex, axis=AX.X)
        rr = stp.tile([64, 1], F32)
        nc.vector.reciprocal(rr, rs_)
        attn = stp.tile([64, 128], F32)
        nc.vector.tensor_scalar_mul(attn, ex, rr)

        atp = ps_work.tile([128, 64], F32, tag="stx", bufs=1, name="atp")
        nc.tensor.transpose(atp, attn, id_sb)
        attnT = stp.tile([128, 64], F32)
        nc.scalar.copy(attnT, atp)
        aw = []
        for s in range(NS):
            w = stp.tile([128, 64], BF16, tag=f"aw{s}", name=f"awt{s}")
            nc.vector.memset(w, 0.0)
            for h2 in range(NH):
                nc.vector.tensor_copy(
                    w[32 * h2:32 * h2 + 32, 16 * h2:16 * h2 + 16],
                    attnT[32 * s:32 * s + 32, 16 * h2:16 * h2 + 16])
            aw.append(w)

        # ---------------- pass 2: V conv + attn@V + outconv + BN/ReLU ----------------
        ps1.close()
        ps2 = ctx.enter_context(tc.tile_pool(name="ps2", bufs=2, space="PSUM"))
        for ch in range(NCH):
            xg, cen_t = load_chunk(ch)
            for q in range(CH // 512):
                fs = 512 * q
                op = ps2.tile([64, 512], F32, tag="op")
                for s in range(NS):
                    vp = ps2.tile([128, 512], F32, tag="vp")
                    nc.tensor.matmul(vp, lhsT=wv_sb[s], rhs=xg[s][:, 2 * q:2 * q + 2, :],
                                     start=True, stop=False, skip_group_check=True)
                    nc.tensor.matmul(vp, lhsT=wvc_sb[s], rhs=cen_t[:, 2 * q:2 * q + 2, :],
                                     start=False, stop=True, skip_group_check=True)
                    vsb = vsb_p.tile([128, 512], BF16)
                    nc.vector.tensor_copy(vsb, vp)
                    nc.tensor.matmul(op, lhsT=aw[s], rhs=vsb,
                                     start=(s == 0), stop=(s == 3))
                osb = osb_p.tile([64, 512], BF16)
                nc.scalar.copy(osb, op)
                fp = ps2.tile([16, 512], F32, tag="fp")
                nc.tensor.matmul(fp, lhsT=wo_sb, rhs=osb, start=True, stop=True)
                fout = fout_p.tile([16, 512], F16)
                nc.scalar.activation(fout, fp, AF.Relu, bias=bnb_sb)
                nc.sync.dma_start(out=out_f[:, ch * CH + fs:ch * CH + fs + 512],
                                  in_=fout)
    return nc


_RT = None


def _get_rt():
    """Build the Bass program once and wrap it in a cached jitted SPMD
    executable (same machinery as bass_utils.run_bass_kernel_spmd's axon
    path, but the jit wrapper is reused across calls so recompilation
    happens only once per process)."""
    global _RT
    if _RT is not None:
        return _RT
    import jax
    from jax.sharding import Mesh, PartitionSpec
    from jax.experimental.shard_map import shard_map
    from concourse import bass2jax as b2j

    nc = _build_program()
    if not nc.is_finalized():
        nc.finalize()
    b2j.install_neuronx_cc_hook()
    partition_name = nc.partition_id_tensor.name if nc.partition_id_tensor else None
    in_names, out_names, out_avals, zero_shapes = [], [], [], []
    for alloc in nc.m.functions[0].allocations:
        if not isinstance(alloc, mybir.MemoryLocationSet):
            continue
        name = alloc.memorylocations[0].name
        if alloc.kind == "ExternalInput":
            if name != partition_name:
                in_names.append(name)
        elif alloc.kind == "ExternalOutput":
            shape = tuple(alloc.tensor_shape)
            dtype = mybir.dt.np(alloc.dtype)
            out_avals.append(jax.core.ShapedArray(shape, dtype))
            out_names.append(name)
            zero_shapes.append((shape, dtype))
    n_params = len(in_names)
    n_outs = len(out_avals)
    all_in = list(in_names) + list(out_names)
    if partition_name is not None:
        all_in.append(partition_name)
    donate = tuple(range(n_params, n_params + n_outs))

    def _body(*args):
        operands = list(args)
        if partition_name is not None:
            operands.append(b2j.partition_id_tensor())
        outs = b2j._bass_exec_p.bind(
            *operands,
            out_avals=tuple(out_avals),
            in_names=tuple(all_in),
            out_names=tuple(out_names),
            lowering_input_output_aliases=(),
            sim_require_finite=True,
            sim_require_nnan=True,
            nc=nc,
        )
        return tuple(outs)

    devices = jax.devices()[:8]
    mesh = Mesh(np.asarray(devices), ("core",))
    in_specs = (PartitionSpec("core"),) * (n_params + n_outs)
    out_specs = (PartitionSpec("core"),) * n_outs
    sharded = jax.jit(
        shard_map(_body, mesh=mesh, in_specs=in_specs,
                  out_specs=out_specs, check_rep=False),
        donate_argnums=donate, keep_unused=True)
    _RT = (sharded, in_names, out_names, out_avals, zero_shapes)
    return _RT


_WCACHE = {"digest": None, "arrs": None}
_PREV_OUT = [None]


def kernel(cen, q_w, k_w, v_w, out_w, bn_gamma, bn_beta, bn_mean, bn_var):
    import jax
    import hashlib
    from jax.sharding import Mesh, PartitionSpec, NamedSharding

    bf = ml_dtypes.bfloat16
    sharded, in_names, out_names, out_avals, zero_shapes = _get_rt()
    mesh = Mesh(np.asarray(jax.devices()[:8]), ("core",))
    sh = NamedSharding(mesh, PartitionSpec("core"))

    # ---- pad input: build + start async transfer first so it overlaps
    # with the host-side weight prep below.
    padded = np.pad(cen, ((0, 0), (0, 0), (8, 8), (8, 8)), mode="reflect").astype(bf)
    pad_g = np.empty((8, 16, PADR, PADC), bf)
    pad_g[0::2] = padded[:, :, 0:PADR, :]        # cores b*2   (top half)
    pad_g[1::2] = padded[:, :, 128:128 + PADR, :]  # cores b*2+1 (bottom half)
    pad_dev = jax.device_put(pad_g.reshape(8 * 16, PADR, PADC), sh)

    # ---- static/weight inputs: cached on device, keyed by content digest
    hsh = hashlib.blake2b(digest_size=16)
    for a in (q_w, k_w, v_w, out_w, bn_gamma, bn_beta, bn_mean, bn_var):
        hsh.update(np.ascontiguousarray(a).tobytes())
    dig = hsh.hexdigest()
    if _WCACHE["digest"] != dig:
        scale = bn_gamma / np.sqrt(bn_var + 1e-5)
        wo_np = (out_w * scale[:, None]).T.astype(bf)          # [64,16]
        bnb_np = (bn_beta - bn_mean * scale)[:, None].astype(np.float32)
        wq_np = np.zeros((CIN, 64), np.float32)
        for h2 in range(NH):
            for o in range(4):
                for s in range(NS):
                    wq_np[:, 16 * h2 + o * 4 + s] = q_w[s, 4 * h2 + o, :]
        wq_np = wq_np.astype(bf)
        wk_np = np.ascontiguousarray(np.transpose(k_w, (0, 2, 1))).astype(bf)
        wv_np = np.ascontiguousarray(np.transpose(v_w, (0, 2, 1))).astype(bf)
        # folded center-subtraction weights: sur = shift - cen, so
        # K = Wk@shift + (-sum_j Wk_j)@cen; same for V.
        wkc_np = np.concatenate(
            [-k_w[s].reshape(128, 8, 16).sum(axis=1).T for s in range(NS)],
            axis=1).astype(bf)                                  # [16, 512]
        wvc_np = np.stack(
            [-v_w[s].reshape(128, 8, 16).sum(axis=1).T for s in range(NS)]).astype(bf)
        oblk = np.zeros((64, 4), np.float32)
        for h2 in range(NH):
            oblk[16 * h2:16 * (h2 + 1), h2] = 1.0
        oblk = oblk.astype(bf)
        ident = np.eye(64, dtype=np.float32)
        oblkt = np.ascontiguousarray(oblk.astype(np.float32).T)
        pms = []
        for core in range(8):
            pm = np.zeros((65, 8), np.float32)
            pm[:, 2 * (core // 2):2 * (core // 2) + 2] = 1.0
            pms.append(pm)
        vals = dict(wk=wk_np, wv=wv_np, wkc=wkc_np, wvc=wvc_np, wq=wq_np,
                    wo=wo_np, bnb=bnb_np, onesblk=oblk, ident=ident,
                    oblkt=oblkt, pmask=None)
        arrs = {}
        for name in in_names:
            if name == "pad":
                continue
            if name == "pmask":
                g = np.concatenate(pms, axis=0)
            else:
                g = np.concatenate([vals[name]] * 8, axis=0)
            arrs[name] = jax.device_put(g, sh)
        _WCACHE["digest"] = dig
        _WCACHE["arrs"] = arrs
    arrs = _WCACHE["arrs"]

    # ---- donated output buffers: the kernel writes every element, so the
    # donated buffer's content is irrelevant — reuse the previous call's
    # output buffer instead of shipping fresh zeros over the wire.
    if _PREV_OUT[0] is not None:
        zargs = _PREV_OUT[0]
    else:
        zargs = [jax.device_put(
            np.zeros((8 * shape[0], *shape[1:]), dtype), sh)
            for shape, dtype in zero_shapes]

    args = [pad_dev if n == "pad" else arrs[n] for n in in_names]
    out_arrs = sharded(*args, *zargs)
    oi = out_names.index("out")
    res = np.asarray(out_arrs[oi]).reshape(8, 16, ROWS, W)
    _PREV_OUT[0] = list(out_arrs)

    out = np.empty((B, 16, H, W), np.float32)
    for core in range(8):
        b, half = core // 2, core % 2
        out[b, :, half * 128:half * 128 + 128, :] = res[core].astype(np.float32)
    return out
